# revision 24
# baseline (speedup 1.0000x reference)
"""Bayesian dense layer (per-sample reparameterized weights) on 8 TRN2 NeuronCores.

Computes out[b] = x[b] @ (W[b] * softplus(log_std) + mean) + bias for
B=512, IN=OUT=1024, data-parallel over the batch axis (64 rows per core).

Final design (measured ~329 us/core; f32 baseline was 829 us):
  - W is uploaded per-tile in a mix of bf16 (60 tiles) and fp8e3 (68 tiles;
    e3m4 has 4 mantissa bits, pre-scaled x32 on host so the Gaussian values
    sit in its normal range; the matching stationary x column is divided by
    32 on device, an exact bf16 exponent shift).  HBM W traffic: ~99 MB/core
    vs 268 MB for f32.  Total rel err ~1.1e-2 (budget 2e-2).
  - fp8 tiles are converted to bf16 by the ACT engine (scalar.copy); the
    S multiply always runs on DVE as bf16 tensor_tensor in 2x packed mode,
    split into FD=2048 halves (measured 1212 ns each; FD=4096 ops run ~25%
    over model).  GpSimd is deliberately unused: any GP op blocks concurrent
    DVE tensor_tensor on the shared SBUF port pair (measured 3-4x DVE
    inflation), so GP offload is a net loss.
  - Batch rows are processed in groups of 4 with column-tiled matmuls
    (tile_position=(0,32j)): the four rows' N=512 matmuls run concurrently
    in disjoint 32-column groups of the PE array (measured quartet issue
    gaps ~8 ns), and their PSUM accumulators live at partitions
    {0,32,64,96} of one [128, OUT] psum tile, so a group drains with a
    single ACT copy + a single 4-row scatter DMA.
  - Ring discipline: 1 MiB/512 KiB W-tile DMAs own the sync HWDGE ring;
    singles, mean, scatters and the output ride the scalar HWDGE ring
    (mixing them head-of-line-blocks the W stream).  The mean term is
    emitted two row-groups before the end so its DMA and matmuls hide in
    the pipeline tail; the last two tile sets are bf16-only so no ACT
    convert sits on the critical tail path.

Per-core steady state (measured): DVE 312 us busy (the binding engine:
256 multiply-halves x 1212 ns), ACT ~80%, DMA ~91% at ~335 GB/s effective,
PE ~55%.
"""

import os
import sys

for _p in ("/root/.axon_site", "/root/.axon_site/_ro/trn_rl_repo",
           "/root/.axon_site/_ro/pypackages"):
    if os.path.isdir(_p) and _p not in sys.path:
        sys.path.append(_p)

import numpy as np

import concourse.bass as bass
import concourse.mybir as mybir
import concourse.tile as tile
from concourse import bacc
from concourse.bass_utils import run_bass_kernel_spmd

B, IN, OUT = 512, 1024, 1024
NCORES = 8
BPC = B // NCORES  # batch rows per core
F8SCALE = 32.0     # host premultiplies fp8 tiles by this; x column divided
GRP = 4            # batch rows per column-tiled PE group

_BUILT = {}


def _channels(nsets):
    """Channel per tile, consumed in (set, j) order; one set = 4 tiles of
    consecutive rows b=4g+j at one i-block.  GpSimd is unusable here (any GP
    op blocks concurrent DVE tensor_tensor on the shared SBUF port), so all
    multiplies run on DVE; ACT converts as many fp8 tiles as it can absorb."""
    ch = []
    for s in range(nsets):
        if s >= nsets - 2:
            ch += ["B", "B", "B", "B"]   # convert-free tail
        elif s % 4 == 1:
            ch += ["A", "B", "A", "A"]
        else:
            ch += ["A", "B", "A", "B"]
    return ch


def build_bass(bpc=BPC, in_dim=IN, out_dim=OUT):
    key = (bpc, in_dim, out_dim)
    if key in _BUILT:
        return _BUILT[key]

    f32 = mybir.dt.float32
    bf16 = mybir.dt.bfloat16
    fp8e3 = mybir.dt.float8e3
    jjb = 4
    ibsz = 128 * jjb              # 512
    nib = in_dim // ibsz          # 2
    nch = max(1, out_dim // 512)
    chunk = out_dim // nch
    ngrp = bpc // GRP             # 16 row groups
    nset = ngrp * nib             # 32 tile sets of 4

    ch = _channels(nset)
    nBF = sum(c == "B" for c in ch)
    nF8 = len(ch) - nBF

    nc = bacc.Bacc("TRN2", target_bir_lowering=False, debug=False,
                   num_devices=NCORES)

    xTh = nc.dram_tensor("xTh", [in_dim, bpc], bf16, kind="ExternalInput").ap()
    W_bf = nc.dram_tensor("W_bf", [max(nBF, 1), ibsz, out_dim], bf16,
                          kind="ExternalInput").ap()
    W_f8 = nc.dram_tensor("W_f8", [max(nF8, 1), ibsz, out_dim], fp8e3,
                          kind="ExternalInput").ap()
    S = nc.dram_tensor("S", [in_dim, out_dim], bf16, kind="ExternalInput").ap()
    mean = nc.dram_tensor("mean", [in_dim, out_dim], bf16,
                          kind="ExternalInput").ap()
    bias = nc.dram_tensor("bias", [1, out_dim], f32, kind="ExternalInput").ap()
    out = nc.dram_tensor("out", [bpc, out_dim], f32,
                         kind="ExternalOutput").ap()

    with tile.TileContext(nc) as tc:
        with (
            tc.tile_pool(name="singles", bufs=1) as singles,
            tc.tile_pool(name="wbf", bufs=5) as wbf,
            tc.tile_pool(name="wf8", bufs=12) as wf8,
            tc.tile_pool(name="cpool", bufs=3) as cpool,
            tc.tile_pool(name="hpool", bufs=7) as hpool,
            tc.tile_pool(name="opool", bufs=2) as opool,
            tc.tile_pool(name="psum", bufs=1, space="PSUM") as psum,
            tc.tile_pool(name="psg", bufs=2, space="PSUM") as psg,
        ):
            # singles ride the scalar HWDGE ring so W tiles start immediately
            # on the sync ring
            xTh_sb = singles.tile([128, nib, jjb, bpc], bf16)
            nc.scalar.dma_start(
                out=xTh_sb,
                in_=xTh.rearrange("(ib p jj) b -> p ib jj b", p=128, jj=jjb))
            xT32_sb = singles.tile([128, nib, jjb, bpc], bf16)
            nc.vector.tensor_scalar_mul(xT32_sb, xTh_sb, 1.0 / F8SCALE)
            S_sb = singles.tile([128, nib, jjb, out_dim], bf16)
            Sr = S.rearrange("(ib p jj) o -> ib p jj o", p=128, jj=jjb)
            for ib in range(nib):  # split so the first multiply starts early
                nc.scalar.dma_start(out=S_sb[:, ib], in_=Sr[ib])
            bias_sb = singles.tile([1, out_dim], f32)
            nc.scalar.dma_start(out=bias_sb, in_=bias)
            ones = singles.tile([1, bpc], f32)
            nc.vector.memset(ones, 1.0)

            # mean term at full PE width: mb = xTh.T @ mean + bias.  Emitted
            # two groups before the end so its DMA and matmuls hide inside
            # the pipeline instead of serializing the tail.
            mb_sb = singles.tile([bpc, out_dim], f32)

            def emit_mean_term():
                acc_m = psum.tile([bpc, out_dim], f32)
                for ib in range(nib):
                    m_t = wbf.tile([128, jjb, out_dim], bf16, tag="w",
                                   name=f"m_t{ib}")
                    nc.scalar.dma_start(
                        out=m_t,
                        in_=mean[ib * ibsz:(ib + 1) * ibsz, :]
                        .rearrange("(p jj) o -> p jj o", jj=jjb))
                    for jj in range(jjb):
                        for n in range(nch):
                            nc.tensor.matmul(
                                acc_m[:, n * chunk:(n + 1) * chunk],
                                xTh_sb[:, ib, jj, :],
                                m_t[:, jj, n * chunk:(n + 1) * chunk],
                                start=(ib == 0 and jj == 0), stop=False,
                                skip_group_check=True)
                for n in range(nch):
                    nc.tensor.matmul(
                        acc_m[:, n * chunk:(n + 1) * chunk],
                        ones,
                        bias_sb[:, n * chunk:(n + 1) * chunk],
                        start=False, stop=True, skip_group_check=True)
                nc.scalar.copy(mb_sb, acc_m)

            # ── per-sample term, 4 rows per column-tiled PE group ──
            wt_sb = singles.tile([bpc, out_dim], f32)
            kB = kF = 0
            t = 0
            for g in range(ngrp):
                if g == ngrp - 2:
                    emit_mean_term()
                acc4 = psg.tile([128, out_dim], f32, tag="acc",
                                name=f"acc{g}")
                stats = [None] * (nib * GRP)
                whs = [None] * (nib * GRP)
                for ib in range(nib):
                    for j in range(GRP):
                        c = ch[t]
                        t += 1
                        if c == "B":
                            w_t = wbf.tile([128, jjb, out_dim], bf16, tag="w",
                                           name=f"w_t{g}_{ib}_{j}")
                            nc.sync.dma_start(
                                out=w_t,
                                in_=W_bf[kB].rearrange(
                                    "(p jj) o -> p jj o", jj=jjb))
                            kB += 1
                            w_h = hpool.tile([128, jjb, out_dim], bf16,
                                             tag="wh", name=f"w_h{g}_{ib}_{j}")
                            for hf in range(2):
                                sl2 = slice(2 * hf, 2 * hf + 2)
                                nc.vector.tensor_mul(
                                    w_h[:, sl2], w_t[:, sl2],
                                    S_sb[:, ib, sl2])
                            stat = xTh_sb
                        else:
                            w8 = wf8.tile([128, jjb, out_dim], fp8e3,
                                          tag="w8", name=f"w8_{g}_{ib}_{j}")
                            nc.sync.dma_start(
                                out=w8,
                                in_=W_f8[kF].rearrange(
                                    "(p jj) o -> p jj o", jj=jjb))
                            kF += 1
                            w_h = hpool.tile([128, jjb, out_dim], bf16,
                                             tag="wh", name=f"w_h{g}_{ib}_{j}")
                            if c == "A":
                                w_c = cpool.tile([128, jjb, out_dim], bf16,
                                                 tag="wc",
                                                 name=f"w_c{g}_{ib}_{j}")
                                nc.scalar.copy(w_c, w8)
                                for hf in range(2):
                                    sl2 = slice(2 * hf, 2 * hf + 2)
                                    nc.vector.tensor_mul(
                                        w_h[:, sl2], w_c[:, sl2],
                                        S_sb[:, ib, sl2])
                            else:  # G
                                nc.gpsimd.tensor_mul(w_h, w8, S_sb[:, ib])
                            stat = xT32_sb
                        stats[ib * GRP + j] = stat
                        whs[ib * GRP + j] = w_h
                    # issue the 4 rows' matmuls j-innermost so they run
                    # concurrently in disjoint 32-column groups
                    for jj in range(jjb):
                        for n in range(nch):
                            for j in range(GRP):
                                b = g * GRP + j
                                nc.tensor.matmul(
                                    acc4[32 * j:32 * j + 1,
                                         n * chunk:(n + 1) * chunk],
                                    stats[ib * GRP + j][:, ib, jj, b:b + 1],
                                    whs[ib * GRP + j][:, jj,
                                                      n * chunk:(n + 1) * chunk],
                                    start=(ib == 0 and jj == 0),
                                    stop=(ib == nib - 1 and jj == jjb - 1),
                                    tile_position=(0, 32 * j),
                                    skip_group_check=True)
                # drain all 4 rows with one ACT copy + one scatter DMA
                col4 = opool.tile([128, out_dim], f32, tag="col",
                                  name=f"col{g}")
                nc.scalar.copy(col4, acc4)
                nc.scalar.dma_start(
                    out=wt_sb[g * GRP:(g + 1) * GRP, :],
                    in_=col4.rearrange("(j q) o -> j q o", j=GRP)[:, 0])

            # ── merge and write out ──
            nc.vector.tensor_add(wt_sb, wt_sb, mb_sb)
            nc.scalar.dma_start(out=out, in_=wt_sb)

    nc.finalize()
    _BUILT[key] = nc
    return nc


def _softplus(x):
    return np.logaddexp(0.0, x.astype(np.float32)).astype(np.float32)


def _run(x, W, mean, log_std, bias, **kwargs):
    import ml_dtypes
    bf16 = ml_dtypes.bfloat16
    fp8 = ml_dtypes.float8_e3m4
    jjb = 4
    ibsz = 128 * jjb
    nib = IN // ibsz
    ngrp = BPC // GRP
    ch = _channels(ngrp * nib)

    x = np.ascontiguousarray(x, dtype=np.float32)
    W = np.ascontiguousarray(W, dtype=np.float32)
    mean_h = np.ascontiguousarray(mean, dtype=np.float32).astype(bf16)
    bias2 = np.ascontiguousarray(bias, dtype=np.float32).reshape(1, OUT)
    S = _softplus(log_std).astype(bf16)

    nc = build_bass()
    in_maps = []
    for cix in range(NCORES):
        sl = slice(cix * BPC, (cix + 1) * BPC)
        Wc = W[sl]  # [BPC, IN, OUT] f32
        bf_tiles, f8_tiles = [], []
        ti = 0
        for g in range(ngrp):
            for ib in range(nib):
                for j in range(GRP):
                    tile_np = Wc[g * GRP + j, ib * ibsz:(ib + 1) * ibsz, :]
                    if ch[ti] == "B":
                        bf_tiles.append(tile_np.astype(bf16))
                    else:
                        f8_tiles.append((tile_np * F8SCALE).astype(fp8))
                    ti += 1
        in_maps.append({
            "xTh": np.ascontiguousarray(x[sl].T).astype(bf16),
            "W_bf": np.stack(bf_tiles),
            "W_f8": np.stack(f8_tiles),
            "S": S,
            "mean": mean_h,
            "bias": bias2,
        })
    res = run_bass_kernel_spmd(nc, in_maps, core_ids=list(range(NCORES)),
                               **kwargs)
    out = np.concatenate([res.results[c]["out"] for c in range(NCORES)],
                         axis=0)
    return out, res


def kernel(x, W, mean, log_std, bias):
    return _run(x, W, mean, log_std, bias)[0]


# revision 26
# speedup vs baseline: 1.1866x; 1.1866x over previous
"""Bayesian dense layer (per-sample reparameterized weights) on 8 TRN2 NeuronCores.

Computes out[b] = x[b] @ (W[b] * softplus(log_std) + mean) + bias for
B=512, IN=OUT=1024, data-parallel over the batch axis (64 rows per core).

Final design (measured ~329 us/core on HW; the f32 baseline was 829 us):
  - W is uploaded per-tile in a mix of bf16 (60 tiles) and fp8e3 (68 tiles;
    e3m4 has 4 mantissa bits, pre-scaled x32 on host so the Gaussian values
    sit in its normal range; the matching stationary x column is divided by
    32 on device, an exact bf16 exponent shift).  HBM W traffic: ~99 MB/core
    vs 268 MB for f32.  Total rel err ~1.1e-2 (budget 2e-2; e4m3 would fail
    at 2.2e-2).
  - fp8 tiles are converted to bf16 by the ACT engine (scalar.copy); the
    S multiply always runs on DVE as bf16 tensor_tensor in 2x packed mode,
    split into FD=2048 halves (measured 1212 ns each; FD=4096 ops run ~25%
    over model).  GpSimd is deliberately unused: any GP op blocks concurrent
    DVE tensor_tensor on the shared SBUF port pair (measured 3-4x DVE
    inflation), so GP offload is a net loss.
  - Batch rows are processed in groups of 4 with column-tiled matmuls
    (tile_position=(0,32j)): the four rows' N=512 matmuls run concurrently
    in disjoint 32-column groups of the PE array (measured quartet issue
    gaps ~8 ns), and their PSUM accumulators live at partitions
    {0,32,64,96} of one [128, OUT] psum tile, so a group drains with a
    single ACT copy + a single 4-row scatter DMA.
  - Ring discipline: 1 MiB/512 KiB W-tile DMAs own the sync HWDGE ring;
    singles, mean, scatters and the output ride the scalar HWDGE ring
    (mixing them head-of-line-blocks the W stream).  The mean term is
    emitted two row-groups before the end so its DMA and matmuls hide in
    the pipeline tail; the last two tile sets are bf16-only so no ACT
    convert sits on the critical tail path.

Per-core steady state (measured): DVE 312 us busy (the binding engine:
256 multiply-halves x 1212 ns), ACT ~80%, DMA ~91% at ~335 GB/s effective,
PE ~55%.
"""

import os
import sys

for _p in ("/root/.axon_site", "/root/.axon_site/_ro/trn_rl_repo",
           "/root/.axon_site/_ro/pypackages"):
    if os.path.isdir(_p) and _p not in sys.path:
        sys.path.append(_p)

import numpy as np

import concourse.bass as bass
import concourse.mybir as mybir
import concourse.tile as tile
from concourse import bacc
from concourse.bass_utils import run_bass_kernel_spmd

B, IN, OUT = 512, 1024, 1024
NCORES = 8
BPC = B // NCORES  # batch rows per core
F8SCALE = 32.0     # host premultiplies fp8 tiles by this; x column divided
GRP = 4            # batch rows per column-tiled PE group

_BUILT = {}


def _channels(nsets):
    """Channel per tile, consumed in (set, j) order; one set = 4 tiles of
    consecutive rows b=4g+j at one i-block.  GpSimd is unusable here (any GP
    op blocks concurrent DVE tensor_tensor on the shared SBUF port), so all
    multiplies run on DVE; ACT converts as many fp8 tiles as it can absorb."""
    ch = []
    for s in range(nsets):
        if s >= nsets - 2:
            ch += ["B", "B", "B", "B"]   # convert-free tail
        elif s % 4 == 1:
            ch += ["A", "B", "A", "A"]
        else:
            ch += ["A", "B", "A", "B"]
    return ch


def build_bass(bpc=BPC, in_dim=IN, out_dim=OUT):
    key = (bpc, in_dim, out_dim)
    if key in _BUILT:
        return _BUILT[key]

    f32 = mybir.dt.float32
    bf16 = mybir.dt.bfloat16
    fp8e3 = mybir.dt.float8e3
    jjb = 4
    ibsz = 128 * jjb              # 512
    nib = in_dim // ibsz          # 2
    nch = max(1, out_dim // 512)
    chunk = out_dim // nch
    ngrp = bpc // GRP             # 16 row groups
    nset = ngrp * nib             # 32 tile sets of 4

    ch = _channels(nset)
    nBF = sum(c == "B" for c in ch)
    nF8 = len(ch) - nBF

    nc = bacc.Bacc("TRN2", target_bir_lowering=False, debug=False,
                   num_devices=NCORES)

    xTh = nc.dram_tensor("xTh", [in_dim, bpc], bf16, kind="ExternalInput").ap()
    W_bf = nc.dram_tensor("W_bf", [max(nBF, 1), ibsz, out_dim], bf16,
                          kind="ExternalInput").ap()
    W_f8 = nc.dram_tensor("W_f8", [max(nF8, 1), ibsz, out_dim], fp8e3,
                          kind="ExternalInput").ap()
    S = nc.dram_tensor("S", [in_dim, out_dim], bf16, kind="ExternalInput").ap()
    mean = nc.dram_tensor("mean", [in_dim, out_dim], bf16,
                          kind="ExternalInput").ap()
    bias = nc.dram_tensor("bias", [1, out_dim], f32, kind="ExternalInput").ap()
    out = nc.dram_tensor("out", [bpc, out_dim], f32,
                         kind="ExternalOutput").ap()

    with tile.TileContext(nc) as tc:
        with (
            tc.tile_pool(name="singles", bufs=1) as singles,
            tc.tile_pool(name="wbf", bufs=5) as wbf,
            tc.tile_pool(name="wf8", bufs=12) as wf8,
            tc.tile_pool(name="cpool", bufs=3) as cpool,
            tc.tile_pool(name="hpool", bufs=7) as hpool,
            tc.tile_pool(name="opool", bufs=2) as opool,
            tc.tile_pool(name="psum", bufs=1, space="PSUM") as psum,
            tc.tile_pool(name="psg", bufs=2, space="PSUM") as psg,
        ):
            # singles ride the scalar HWDGE ring so W tiles start immediately
            # on the sync ring
            xTh_sb = singles.tile([128, nib, jjb, bpc], bf16)
            nc.scalar.dma_start(
                out=xTh_sb,
                in_=xTh.rearrange("(ib p jj) b -> p ib jj b", p=128, jj=jjb))
            xT32_sb = singles.tile([128, nib, jjb, bpc], bf16)
            nc.vector.tensor_scalar_mul(xT32_sb, xTh_sb, 1.0 / F8SCALE)
            S_sb = singles.tile([128, nib, jjb, out_dim], bf16)
            nc.scalar.dma_start(
                out=S_sb,
                in_=S.rearrange("(ib p jj) o -> p ib jj o", p=128, jj=jjb))
            bias_sb = singles.tile([1, out_dim], f32)
            nc.scalar.dma_start(out=bias_sb, in_=bias)
            ones = singles.tile([1, bpc], f32)
            nc.vector.memset(ones, 1.0)

            # mean term at full PE width: mb = xTh.T @ mean + bias.  Emitted
            # two groups before the end so its DMA and matmuls hide inside
            # the pipeline instead of serializing the tail.
            mb_sb = singles.tile([bpc, out_dim], f32)

            def emit_mean_term():
                acc_m = psum.tile([bpc, out_dim], f32)
                for ib in range(nib):
                    m_t = wbf.tile([128, jjb, out_dim], bf16, tag="w",
                                   name=f"m_t{ib}")
                    nc.scalar.dma_start(
                        out=m_t,
                        in_=mean[ib * ibsz:(ib + 1) * ibsz, :]
                        .rearrange("(p jj) o -> p jj o", jj=jjb))
                    for jj in range(jjb):
                        for n in range(nch):
                            nc.tensor.matmul(
                                acc_m[:, n * chunk:(n + 1) * chunk],
                                xTh_sb[:, ib, jj, :],
                                m_t[:, jj, n * chunk:(n + 1) * chunk],
                                start=(ib == 0 and jj == 0), stop=False,
                                skip_group_check=True)
                for n in range(nch):
                    nc.tensor.matmul(
                        acc_m[:, n * chunk:(n + 1) * chunk],
                        ones,
                        bias_sb[:, n * chunk:(n + 1) * chunk],
                        start=False, stop=True, skip_group_check=True)
                nc.scalar.copy(mb_sb, acc_m)

            # ── per-sample term, 4 rows per column-tiled PE group ──
            wt_sb = singles.tile([bpc, out_dim], f32)
            kB = kF = 0
            t = 0
            for g in range(ngrp):
                if g == ngrp - 2:
                    emit_mean_term()
                acc4 = psg.tile([128, out_dim], f32, tag="acc",
                                name=f"acc{g}")
                stats = [None] * (nib * GRP)
                whs = [None] * (nib * GRP)
                for ib in range(nib):
                    for j in range(GRP):
                        c = ch[t]
                        t += 1
                        if c == "B":
                            w_t = wbf.tile([128, jjb, out_dim], bf16, tag="w",
                                           name=f"w_t{g}_{ib}_{j}")
                            nc.sync.dma_start(
                                out=w_t,
                                in_=W_bf[kB].rearrange(
                                    "(p jj) o -> p jj o", jj=jjb))
                            kB += 1
                            w_h = hpool.tile([128, jjb, out_dim], bf16,
                                             tag="wh", name=f"w_h{g}_{ib}_{j}")
                            for hf in range(2):
                                sl2 = slice(2 * hf, 2 * hf + 2)
                                nc.vector.tensor_mul(
                                    w_h[:, sl2], w_t[:, sl2],
                                    S_sb[:, ib, sl2])
                            stat = xTh_sb
                        else:
                            w8 = wf8.tile([128, jjb, out_dim], fp8e3,
                                          tag="w8", name=f"w8_{g}_{ib}_{j}")
                            nc.sync.dma_start(
                                out=w8,
                                in_=W_f8[kF].rearrange(
                                    "(p jj) o -> p jj o", jj=jjb))
                            kF += 1
                            w_h = hpool.tile([128, jjb, out_dim], bf16,
                                             tag="wh", name=f"w_h{g}_{ib}_{j}")
                            if c == "A":
                                w_c = cpool.tile([128, jjb, out_dim], bf16,
                                                 tag="wc",
                                                 name=f"w_c{g}_{ib}_{j}")
                                nc.scalar.copy(w_c, w8)
                                for hf in range(2):
                                    sl2 = slice(2 * hf, 2 * hf + 2)
                                    nc.vector.tensor_mul(
                                        w_h[:, sl2], w_c[:, sl2],
                                        S_sb[:, ib, sl2])
                            else:  # G
                                nc.gpsimd.tensor_mul(w_h, w8, S_sb[:, ib])
                            stat = xT32_sb
                        stats[ib * GRP + j] = stat
                        whs[ib * GRP + j] = w_h
                    # issue the 4 rows' matmuls j-innermost so they run
                    # concurrently in disjoint 32-column groups
                    for jj in range(jjb):
                        for n in range(nch):
                            for j in range(GRP):
                                b = g * GRP + j
                                nc.tensor.matmul(
                                    acc4[32 * j:32 * j + 1,
                                         n * chunk:(n + 1) * chunk],
                                    stats[ib * GRP + j][:, ib, jj, b:b + 1],
                                    whs[ib * GRP + j][:, jj,
                                                      n * chunk:(n + 1) * chunk],
                                    start=(ib == 0 and jj == 0),
                                    stop=(ib == nib - 1 and jj == jjb - 1),
                                    tile_position=(0, 32 * j),
                                    skip_group_check=True)
                # drain all 4 rows with one ACT copy + one scatter DMA
                col4 = opool.tile([128, out_dim], f32, tag="col",
                                  name=f"col{g}")
                nc.scalar.copy(col4, acc4)
                nc.scalar.dma_start(
                    out=wt_sb[g * GRP:(g + 1) * GRP, :],
                    in_=col4.rearrange("(j q) o -> j q o", j=GRP)[:, 0])

            # ── merge and write out ──
            nc.vector.tensor_add(wt_sb, wt_sb, mb_sb)
            nc.scalar.dma_start(out=out, in_=wt_sb)

    nc.finalize()
    _BUILT[key] = nc
    return nc


def _softplus(x):
    return np.logaddexp(0.0, x.astype(np.float32)).astype(np.float32)


def _run(x, W, mean, log_std, bias, **kwargs):
    import ml_dtypes
    bf16 = ml_dtypes.bfloat16
    fp8 = ml_dtypes.float8_e3m4
    jjb = 4
    ibsz = 128 * jjb
    nib = IN // ibsz
    ngrp = BPC // GRP
    ch = _channels(ngrp * nib)

    x = np.ascontiguousarray(x, dtype=np.float32)
    W = np.ascontiguousarray(W, dtype=np.float32)
    mean_h = np.ascontiguousarray(mean, dtype=np.float32).astype(bf16)
    bias2 = np.ascontiguousarray(bias, dtype=np.float32).reshape(1, OUT)
    S = _softplus(log_std).astype(bf16)

    nc = build_bass()
    in_maps = []
    for cix in range(NCORES):
        sl = slice(cix * BPC, (cix + 1) * BPC)
        Wc = W[sl]  # [BPC, IN, OUT] f32
        bf_tiles, f8_tiles = [], []
        ti = 0
        for g in range(ngrp):
            for ib in range(nib):
                for j in range(GRP):
                    tile_np = Wc[g * GRP + j, ib * ibsz:(ib + 1) * ibsz, :]
                    if ch[ti] == "B":
                        bf_tiles.append(tile_np.astype(bf16))
                    else:
                        f8_tiles.append((tile_np * F8SCALE).astype(fp8))
                    ti += 1
        in_maps.append({
            "xTh": np.ascontiguousarray(x[sl].T).astype(bf16),
            "W_bf": np.stack(bf_tiles),
            "W_f8": np.stack(f8_tiles),
            "S": S,
            "mean": mean_h,
            "bias": bias2,
        })
    res = run_bass_kernel_spmd(nc, in_maps, core_ids=list(range(NCORES)),
                               **kwargs)
    out = np.concatenate([res.results[c]["out"] for c in range(NCORES)],
                         axis=0)
    return out, res


def kernel(x, W, mean, log_std, bias):
    return _run(x, W, mean, log_std, bias)[0]
